# revision 3
# baseline (speedup 1.0000x reference)
"""CharRNN Trainium2 kernel: data-parallel over batch on 8 NeuronCores.

kernel(**inputs) takes the FULL unsharded inputs (as produced by
setup_inputs) and returns the full [128, 1024, 128] float32 logits.
Each core runs 16 batch rows through the full T=1024 tanh recurrence.

Numerics: 16-bit hi/lo bf16 split on both W_hh and h, 3 products
(Whi@hhi + Whi@hlo + Wlo@hhi), f32 PSUM accumulate — identical to the
reference to ~3e-3.

Perf design (instruction-count-bound on the PE at ~42ns per
LDWEIGHTS+MATMUL pair):
  - 33 matmul pairs per step instead of 50:
      1 identity matmul injects xp_hi AND xp_lo (moving N=128, output
        AP broadcast/stride-0 so both halves accumulate into the same
        PSUM columns),
      16 Wlo matmuls (moving h_hi, N=16),
      16 Whi matmuls (moving [h_hi|h_lo] adjacent pair, N=32, stride-0
        output -> both products accumulate into the same columns).
  - h state lives in the hs ring buffers with layout
    [128, FC, NT, {hi,lo}, B] so the merged moving operand is one
    contiguous 32-col slice and fc can read the hi plane.
  - tanh (bf16, critical path) runs per j-tile (4 ACT ops) so the next
    step's i-blocks can start as soon as their h tile is ready; the
    f32 tanh + lo-subtract run off the critical path.
  - Wlo block (needs only h_hi) runs before the Whi block (needs h_lo),
    with the xp/fc filler matmuls placed between the blocks.
"""
import numpy as np

import concourse.bacc as bacc
import concourse.mybir as mybir
from concourse.tile import TileContext
from concourse.masks import make_identity

f32 = mybir.dt.float32
bf16 = mybir.dt.bfloat16

B = 16        # batch rows per core
H = 512
NT = 4        # hidden tiles
V = 128
E = 16
CH = 32       # steps per xp chunk
FC = 8        # steps per fc block
AF = mybir.ActivationFunctionType
ALU = mybir.AluOpType


def build(T: int = 1024):
    assert T % CH == 0
    nc = bacc.Bacc("TRN2", target_bir_lowering=False, debug=False)

    x_tb = nc.declare_dram_parameter("x_tb", [T, B], f32, isOutput=False)
    emb = nc.declare_dram_parameter("emb", [V, E], f32, isOutput=False)
    W_ih = nc.declare_dram_parameter("W_ih", [H, E], f32, isOutput=False)
    W_hh = nc.declare_dram_parameter("W_hh", [H, H], f32, isOutput=False)
    bias = nc.declare_dram_parameter("bias", [1, H], f32, isOutput=False)  # b_ih+b_hh
    W_fc = nc.declare_dram_parameter("W_fc", [V, H], f32, isOutput=False)
    b_fc = nc.declare_dram_parameter("b_fc", [1, V], f32, isOutput=False)
    out = nc.declare_dram_parameter("out", [B, T, V], f32, isOutput=True)

    n_chunks = T // CH

    with TileContext(nc) as tc:
        with (
            tc.tile_pool(name="const", bufs=1) as cpool,
            tc.tile_pool(name="state", bufs=3) as spool,
            tc.tile_pool(name="hs", bufs=1) as hspool,
            tc.tile_pool(name="xp", bufs=1) as xppool,
            tc.tile_pool(name="work", bufs=2) as wkpool,
            tc.tile_pool(name="ps_rec", bufs=2, space="PSUM") as ps_rec,
            tc.tile_pool(name="ps_xp", bufs=2, space="PSUM") as ps_xp,
            tc.tile_pool(name="ps_fc", bufs=2, space="PSUM") as ps_fc,
        ):
            # ---------------- one-time prep ----------------
            ident_f32 = cpool.tile([128, 128], f32, tag="ident")
            make_identity(nc, ident_f32)
            ident_bf = cpool.tile([128, 128], bf16, tag="identb")
            nc.vector.tensor_copy(ident_bf[:, :], ident_f32[:, :])

            # W_hhT tiles [128_i, NT, 128_j] hi/lo bf16, via PE transpose of
            # natural-layout W_hh [j, i].
            w_nat = wkpool.tile([128, NT, H], f32, tag="wnat")  # [j_p, jt, i]
            nc.sync.dma_start(
                w_nat[:, :, :], W_hh.rearrange("(jt p) i -> p jt i", p=128)
            )
            whh_hi = cpool.tile([128, NT, H], bf16, tag="whh_hi")  # [i_p, it, j]
            whh_lo = cpool.tile([128, NT, H], bf16, tag="whh_lo")
            for it in range(NT):
                for jt in range(NT):
                    tp = ps_xp.tile([128, 128], f32, tag="xpp")
                    nc.tensor.transpose(
                        tp[:, :],
                        w_nat[:, jt, it * 128 : (it + 1) * 128],
                        ident_f32[:, :],
                    )
                    nc.vector.tensor_copy(
                        whh_hi[:, it, jt * 128 : (jt + 1) * 128], tp[:, :]
                    )
                    nc.vector.tensor_tensor(
                        whh_lo[:, it, jt * 128 : (jt + 1) * 128],
                        tp[:, :],
                        whh_hi[:, it, jt * 128 : (jt + 1) * 128],
                        ALU.subtract,
                    )

            # M' = emb @ W_ih.T + bias  -> [128_v, H], split hi/lo bf16
            embT = wkpool.tile([E, V], f32, tag="embT")
            nc.sync.dma_start(embT[:, :], emb.rearrange("v e -> e v"))
            wihT = wkpool.tile([E, H], f32, tag="wihT")
            nc.sync.dma_start(wihT[:, :], W_ih.rearrange("h e -> e h"))
            mp_ps = ps_xp.tile([128, H], f32, tag="xpp")
            nc.tensor.matmul(mp_ps[:, :], embT[:, :], wihT[:, :], start=True, stop=True)
            bias_row = wkpool.tile([1, H], f32, tag="biasrow")
            nc.sync.dma_start(bias_row[:, :], bias[:, :])
            bias_bc = wkpool.tile([128, H], f32, tag="biasbc")
            nc.gpsimd.partition_broadcast(bias_bc[:, :], bias_row[:, :])
            mprime = cpool.tile([128, H], f32, tag="mprime")
            nc.vector.tensor_tensor(mprime[:, :], mp_ps[:, :], bias_bc[:, :], ALU.add)
            mp_hi = cpool.tile([128, H], bf16, tag="mp_hi")
            mp_lo = cpool.tile([128, H], bf16, tag="mp_lo")
            nc.vector.tensor_copy(mp_hi[:, :], mprime[:, :])
            nc.vector.tensor_tensor(mp_lo[:, :], mprime[:, :], mp_hi[:, :], ALU.subtract)

            # W_fcT tiles [128_j, NT, 128_v] bf16 via PE transpose
            wfc_nat = wkpool.tile([128, H], f32, tag="wfcnat")  # [v_p, j]
            nc.sync.dma_start(wfc_nat[:, :], W_fc[:, :])
            wfcT = cpool.tile([128, NT, V], bf16, tag="wfcT")
            for jt in range(NT):
                tp = ps_xp.tile([128, 128], f32, tag="xpp")
                nc.tensor.transpose(
                    tp[:, :], wfc_nat[:, jt * 128 : (jt + 1) * 128], ident_f32[:, :]
                )
                nc.vector.tensor_copy(wfcT[:, jt, :], tp[:, :])

            # b_fc broadcast [128_tok, V]
            bfc_row = wkpool.tile([1, V], f32, tag="bfcrow")
            nc.sync.dma_start(bfc_row[:, :], b_fc[:, :])
            bfc_bc = cpool.tile([128, V], f32, tag="bfcbc")
            nc.gpsimd.partition_broadcast(bfc_bc[:, :], bfc_row[:, :])

            # iota column [128, 1] for onehot compares
            iota_col = cpool.tile([128, 1], f32, tag="iota")
            nc.gpsimd.iota(iota_col[:, :], pattern=[[0, 1]], channel_multiplier=1,
                           allow_small_or_imprecise_dtypes=True)

            # hs ring buffers: [128, FC, NT, 2, B]; plane g=0 is h_hi,
            # g=1 is h_lo.  Slice [:, slot, it, :, :] is the contiguous
            # 32-col [h_hi|h_lo] moving operand for i-tile `it`.
            n_hs = 3
            hsbufs = [
                hspool.tile([128, FC, NT, 2, B], bf16, tag=f"hs{k}", name=f"hs{k}")
                for k in range(n_hs)
            ]
            # initial state (read as "step -1" = slot FC-1 of hsbufs[2])
            nc.vector.memset(
                hsbufs[n_hs - 1][:, FC - 1, :, :, :].rearrange("p a b c -> p (a b c)"),
                0.0,
            )

            # xp chunk double buffers [128, CH, 2, NT*B] bf16:
            # (step-in-chunk, {hi,lo}, jt*B+b).  Slice [:, si, :, :] is the
            # 128-col moving operand of the identity inject matmul.
            xp_bufs = [
                xppool.tile([128, CH, 2, NT * B], bf16, tag=f"xp{par}",
                            name=f"xp{par}")
                for par in range(2)
            ]

            onehot_cur = [None]

            def xp_prep(c):
                """Build onehot for chunk c (off the PE chain)."""
                xrow = wkpool.tile([1, CH * B], f32, tag="xrow")
                nc.sync.dma_start(
                    xrow[:, :],
                    x_tb.rearrange("(a t) b -> a (t b)", t=CH)[c : c + 1, :],
                )
                xbc = wkpool.tile([128, CH * B], f32, tag="xbc")
                nc.gpsimd.partition_broadcast(xbc[:, :], xrow[:, :])
                onehot = wkpool.tile([128, CH * B], bf16, tag="onehot")
                nc.vector.tensor_scalar(
                    onehot[:, :], xbc[:, :], iota_col[:, :], None, ALU.is_equal
                )
                onehot_cur[0] = onehot

            xp_psum_pend = {}

            def xp_mm(c, jt):
                """xp matmul for j-tile jt of chunk c: one psum group with
                both mp_hi and mp_lo product is NOT possible (different
                stationary) -> two matmuls accumulating into one psum."""
                onehot = onehot_cur[0]
                ps = ps_xp.tile([128, CH * B], f32, tag="xpp")
                nc.tensor.matmul(
                    ps[:, :], mp_hi[:, jt * 128 : (jt + 1) * 128], onehot[:, :],
                    start=True, stop=False, skip_group_check=True,
                )
                xp_psum_pend[jt] = ps
                return ps

            def xp_mm2(c, jt):
                ps = xp_psum_pend[jt]
                onehot = onehot_cur[0]
                nc.tensor.matmul(
                    ps[:, :], mp_lo[:, jt * 128 : (jt + 1) * 128], onehot[:, :],
                    start=False, stop=True, skip_group_check=True,
                )

            def xp_scatter(c, jt, piece):
                """Split psum for j-tile jt into bf16 hi/lo planes of the
                xp buffer (8 steps per piece)."""
                par = c % 2
                ps = xp_psum_pend[jt]
                q = CH // 4
                sl = slice(piece * q, (piece + 1) * q)
                dst = xp_bufs[par]
                dh = dst[:, sl, 0, jt * B : (jt + 1) * B]
                dl = dst[:, sl, 1, jt * B : (jt + 1) * B]
                sps = ps.rearrange("p (s b) -> p s b", s=CH)[:, sl, :]
                nc.vector.tensor_copy(dh, sps)
                nc.vector.tensor_tensor(dl, sps, dh, ALU.subtract)

            fc_state = {}

            def fc_part(hsbuf, s0, phase):
                """logits for FC steps starting at s0 from hsbuf; phases
                -1/0 repack (hi+lo add) + first 2 matmuls, 1 finishes."""
                hs_src = hsbuf
                if phase == -1:
                    fcbuf = wkpool.tile([128, NT * 128], bf16, tag="fcbuf")
                    fc_state[("buf", s0)] = fcbuf
                    nc.vector.tensor_tensor(
                        fcbuf.rearrange("p (g b s) -> p s g b", g=NT, b=B)[:, :, 0:2, :],
                        hs_src[:, :, 0:2, 0, :],
                        hs_src[:, :, 0:2, 1, :],
                        ALU.add,
                    )
                    return
                if phase == 0:
                    fcbuf = fc_state.pop(("buf", s0))
                    nc.vector.tensor_tensor(
                        fcbuf.rearrange("p (g b s) -> p s g b", g=NT, b=B)[:, :, 2:4, :],
                        hs_src[:, :, 2:4, 0, :],
                        hs_src[:, :, 2:4, 1, :],
                        ALU.add,
                    )
                    ps = ps_fc.tile([128, V], f32, tag="fcp")
                    fc_state[s0] = (fcbuf, ps)
                    for jt in (0, 1):
                        nc.tensor.matmul(
                            ps[:, :], fcbuf[:, jt * 128 : (jt + 1) * 128],
                            wfcT[:, jt, :],
                            start=(jt == 0), stop=False,
                            skip_group_check=(jt != 0),
                        )
                else:
                    fcbuf, ps = fc_state.pop(s0)
                    for jt in (2, 3):
                        nc.tensor.matmul(
                            ps[:, :], fcbuf[:, jt * 128 : (jt + 1) * 128],
                            wfcT[:, jt, :],
                            start=False, stop=(jt == 3),
                            skip_group_check=(jt != 3),
                        )
                    lg = wkpool.tile([128, V], f32, tag="logits")
                    nc.vector.tensor_tensor(lg[:, :], ps[:, :], bfc_bc[:, :], ALU.add)
                    # out[b, s0+s, v]: partitions p = b*FC + s
                    nc.sync.dma_start(out[:, s0 : s0 + FC, :], lg[:, :])

            def fc_phase(hsbuf, s0):
                fc_part(hsbuf, s0, -1)
                fc_part(hsbuf, s0, 0)
                fc_part(hsbuf, s0, 1)

            def prev_slice(s):
                if s == 0:
                    return hsbufs[n_hs - 1], FC - 1
                return hsbufs[((s - 1) // FC) % n_hs], (s - 1) % FC

            def rec_step_mms(c, s, psum, part):
                """Matmul pairs of step s.  part 0: ident + Wlo block;
                part 1: Whi block."""
                par = c % 2
                si = s - c * CH
                pb, pslot = prev_slice(s)
                if part == 0:
                    # identity inject of xp hi+lo: moving [128, (2,64)],
                    # stride-0 out over the hi/lo dim.
                    o = psum.rearrange("p (o c) -> p o c", o=1).broadcast_to(
                        [128, 2, NT * B])
                    nc.tensor.matmul(
                        o, ident_bf[:, :], xp_bufs[par][:, si, :, :],
                        start=True, stop=False, skip_group_check=True,
                    )
                    for it in range(NT):
                        mov = pb[:, pslot, it, 0, :]          # h_hi, 16 cols
                        for jt in range(NT):
                            nc.tensor.matmul(
                                psum[:, jt * B : (jt + 1) * B],
                                whh_lo[:, it, jt * 128 : (jt + 1) * 128],
                                mov,
                                start=False, stop=False, skip_group_check=True,
                            )
                else:
                    for it in range(NT):
                        mov = pb[:, pslot, it, :, :]          # [h_hi|h_lo], 32
                        for jt in range(NT):
                            last = (it == NT - 1) and (jt == NT - 1)
                            o = psum[:, jt * B : (jt + 1) * B].rearrange(
                                "p (o b) -> p o b", o=1).broadcast_to([128, 2, B])
                            nc.tensor.matmul(
                                o,
                                whh_hi[:, it, jt * 128 : (jt + 1) * 128],
                                mov,
                                start=False, stop=last,
                                skip_group_check=not last,
                            )

            def rec_step_tail(s, psum):
                """tanh chain: per-jt bf16 tanh (critical), then f32 tanh +
                lo subtract (off critical path)."""
                hb = hsbufs[(s // FC) % n_hs]
                slot = s % FC
                for jt in range(NT):
                    nc.scalar.activation(
                        hb[:, slot, jt, 0, :],
                        psum[:, jt * B : (jt + 1) * B],
                        AF.Tanh,
                    )
                hT = spool.tile([128, NT * B], f32, tag="hT")
                nc.scalar.activation(hT[:, :], psum[:, :], AF.Tanh)
                nc.vector.tensor_tensor(
                    hb[:, slot, :, 1, :],
                    hT.rearrange("p (jt b) -> p jt b", jt=NT),
                    hb[:, slot, :, 0, :],
                    ALU.subtract,
                )

            # ---------------- main schedule ----------------
            def fillers(c, s):
                """xp chunk c+1 + fc matmuls, placed between the Wlo and
                Whi blocks of step s."""
                si = s - c * CH
                if c + 1 < n_chunks:
                    if si == 0:
                        xp_prep(c + 1)
                    elif si % 2 == 1 and si < 17:
                        k = si // 2  # 0..7
                        if k % 2 == 0:
                            xp_mm(c + 1, k // 2)
                        else:
                            xp_mm2(c + 1, k // 2)
                    if 4 <= si < 20:
                        jt_s, piece = (si - 4) // 4, (si - 4) % 4
                        if jt_s * 2 + 1 <= (si - 1) // 2:
                            xp_scatter(c + 1, jt_s, piece)
                if (s + 1) % FC == 5 and s + 1 >= FC * 2 - 3:
                    blk = (s + 8) // FC - 2
                    fc_part(hsbufs[blk % n_hs], blk * FC, -1)
                if (s + 1) % FC == 6 and s + 1 >= FC * 2 - 2:
                    blk = (s + 7) // FC - 2
                    fc_part(hsbufs[blk % n_hs], blk * FC, 0)
                if (s + 1) % FC == 0 and s + 1 >= FC * 2:
                    blk = (s + 1) // FC - 2
                    fc_part(hsbufs[blk % n_hs], blk * FC, 1)

            xp_prep(0)
            for jt in range(NT):
                xp_mm(0, jt)
                xp_mm2(0, jt)
                for piece in range(4):
                    xp_scatter(0, jt, piece)

            for c in range(n_chunks):
                for s in range(c * CH, (c + 1) * CH):
                    psum = ps_rec.tile([128, NT * B], f32, tag="rec")
                    rec_step_mms(c, s, psum, 0)
                    fillers(c, s)
                    rec_step_mms(c, s, psum, 1)
                    rec_step_tail(s, psum)
            # final two fc blocks
            for blk in (T // FC - 2, T // FC - 1):
                fc_phase(hsbufs[blk % n_hs], blk * FC)

    nc.finalize()
    return nc


_NC_CACHE = {}


def kernel(x, emb, W_ih, W_hh, b_ih, b_hh, W_fc, b_fc):
    from concourse.bass_utils import run_bass_kernel_spmd

    T_full = 1024
    x = np.asarray(x)
    emb = np.asarray(emb, dtype=np.float32)
    W_ih = np.asarray(W_ih, dtype=np.float32)
    W_hh = np.asarray(W_hh, dtype=np.float32)
    b_ih = np.asarray(b_ih, dtype=np.float32)
    b_hh = np.asarray(b_hh, dtype=np.float32)
    W_fc = np.asarray(W_fc, dtype=np.float32)
    b_fc = np.asarray(b_fc, dtype=np.float32)

    if "nc" not in _NC_CACHE:
        _NC_CACHE["nc"] = build(T_full)
    nc = _NC_CACHE["nc"]

    bias = (b_ih + b_hh).reshape(1, H).astype(np.float32)
    in_maps = []
    for core in range(8):
        xs = x[core * B : (core + 1) * B, :]          # [16, 1024]
        in_maps.append(dict(
            x_tb=np.ascontiguousarray(xs.T).astype(np.float32),
            emb=emb, W_ih=W_ih, W_hh=W_hh, bias=bias,
            W_fc=W_fc, b_fc=b_fc.reshape(1, V),
        ))
    res = run_bass_kernel_spmd(nc, in_maps, core_ids=list(range(8)))
    return np.concatenate([r["out"] for r in res.results], axis=0)


# revision 5
# speedup vs baseline: 1.1494x; 1.1494x over previous
"""CharRNN Trainium2 kernel: data-parallel over batch on 8 NeuronCores.

kernel(**inputs) takes the FULL unsharded inputs (as produced by
setup_inputs) and returns the full [128, 1024, 128] float32 logits.
Each core runs 16 batch rows through the full T=1024 tanh recurrence.

Numerics: 16-bit hi/lo bf16 split on both W_hh and h, 3 products
(Whi@hhi + Whi@hlo + Wlo@hhi), f32 PSUM accumulate — identical to the
reference to ~3e-3.

Perf design (instruction-count-bound on the PE at ~42ns per
LDWEIGHTS+MATMUL pair):
  - 33 matmul pairs per step instead of 50:
      1 identity matmul injects xp_hi AND xp_lo (moving N=128, output
        AP broadcast/stride-0 so both halves accumulate into the same
        PSUM columns),
      16 Wlo matmuls (moving h_hi, N=16),
      16 Whi matmuls (moving [h_hi|h_lo] adjacent pair, N=32, stride-0
        output -> both products accumulate into the same columns).
  - h state lives in the hs ring buffers with layout
    [128, FC, NT, {hi,lo}, B] so the merged moving operand is one
    contiguous 32-col slice and fc can read the hi plane.
  - tanh (bf16, critical path) runs per j-tile (4 ACT ops) so the next
    step's i-blocks can start as soon as their h tile is ready; the
    f32 tanh + lo-subtract run off the critical path.
  - Wlo block (needs only h_hi) runs before the Whi block (needs h_lo),
    with the xp/fc filler matmuls placed between the blocks.
"""
import numpy as np

import concourse.bacc as bacc
import concourse.mybir as mybir
from concourse.tile import TileContext
from concourse.masks import make_identity

f32 = mybir.dt.float32
bf16 = mybir.dt.bfloat16

B = 16        # batch rows per core
H = 512
NT = 4        # hidden tiles
V = 128
E = 16
CH = 32       # steps per xp chunk
FC = 8        # steps per fc block
AF = mybir.ActivationFunctionType
ALU = mybir.AluOpType


def build(T: int = 1024):
    assert T % CH == 0
    nc = bacc.Bacc("TRN2", target_bir_lowering=False, debug=False)

    x_tb = nc.declare_dram_parameter("x_tb", [T, B], f32, isOutput=False)
    emb = nc.declare_dram_parameter("emb", [V, E], f32, isOutput=False)
    W_ih = nc.declare_dram_parameter("W_ih", [H, E], f32, isOutput=False)
    W_hh = nc.declare_dram_parameter("W_hh", [H, H], f32, isOutput=False)
    bias = nc.declare_dram_parameter("bias", [1, H], f32, isOutput=False)  # b_ih+b_hh
    W_fc = nc.declare_dram_parameter("W_fc", [V, H], f32, isOutput=False)
    b_fc = nc.declare_dram_parameter("b_fc", [1, V], f32, isOutput=False)
    out = nc.declare_dram_parameter("out", [B, T, V], f32, isOutput=True)

    n_chunks = T // CH

    with TileContext(nc) as tc:
        with (
            tc.tile_pool(name="const", bufs=1) as cpool,
            tc.tile_pool(name="state", bufs=3) as spool,
            tc.tile_pool(name="hs", bufs=1) as hspool,
            tc.tile_pool(name="xp", bufs=1) as xppool,
            tc.tile_pool(name="work", bufs=2) as wkpool,
            tc.tile_pool(name="ps_rec", bufs=2, space="PSUM") as ps_rec,
            tc.tile_pool(name="ps_xp", bufs=2, space="PSUM") as ps_xp,
            tc.tile_pool(name="ps_fc", bufs=2, space="PSUM") as ps_fc,
        ):
            # ---------------- one-time prep ----------------
            ident_f32 = cpool.tile([128, 128], f32, tag="ident")
            make_identity(nc, ident_f32)
            ident_bf = cpool.tile([128, 128], bf16, tag="identb")
            nc.vector.tensor_copy(ident_bf[:, :], ident_f32[:, :])

            # W_hhT tiles [128_i, NT, 128_j] hi/lo bf16, via PE transpose of
            # natural-layout W_hh [j, i].
            w_nat = wkpool.tile([128, NT, H], f32, tag="wnat")  # [j_p, jt, i]
            nc.sync.dma_start(
                w_nat[:, :, :], W_hh.rearrange("(jt p) i -> p jt i", p=128)
            )
            whh_hi = cpool.tile([128, NT, H], bf16, tag="whh_hi")  # [i_p, it, j]
            whh_lo = cpool.tile([128, NT, H], bf16, tag="whh_lo")
            for it in range(NT):
                for jt in range(NT):
                    tp = ps_xp.tile([128, 128], f32, tag="xpp")
                    nc.tensor.transpose(
                        tp[:, :],
                        w_nat[:, jt, it * 128 : (it + 1) * 128],
                        ident_f32[:, :],
                    )
                    nc.vector.tensor_copy(
                        whh_hi[:, it, jt * 128 : (jt + 1) * 128], tp[:, :]
                    )
                    nc.vector.tensor_tensor(
                        whh_lo[:, it, jt * 128 : (jt + 1) * 128],
                        tp[:, :],
                        whh_hi[:, it, jt * 128 : (jt + 1) * 128],
                        ALU.subtract,
                    )

            # M' = emb @ W_ih.T + bias  -> [128_v, H], split hi/lo bf16
            embT = wkpool.tile([E, V], f32, tag="embT")
            nc.sync.dma_start(embT[:, :], emb.rearrange("v e -> e v"))
            wihT = wkpool.tile([E, H], f32, tag="wihT")
            nc.sync.dma_start(wihT[:, :], W_ih.rearrange("h e -> e h"))
            mp_ps = ps_xp.tile([128, H], f32, tag="xpp")
            nc.tensor.matmul(mp_ps[:, :], embT[:, :], wihT[:, :], start=True, stop=True)
            bias_row = wkpool.tile([1, H], f32, tag="biasrow")
            nc.sync.dma_start(bias_row[:, :], bias[:, :])
            bias_bc = wkpool.tile([128, H], f32, tag="biasbc")
            nc.gpsimd.partition_broadcast(bias_bc[:, :], bias_row[:, :])
            mprime = cpool.tile([128, H], f32, tag="mprime")
            nc.vector.tensor_tensor(mprime[:, :], mp_ps[:, :], bias_bc[:, :], ALU.add)
            mp_hi = cpool.tile([128, H], bf16, tag="mp_hi")
            mp_lo = cpool.tile([128, H], bf16, tag="mp_lo")
            nc.vector.tensor_copy(mp_hi[:, :], mprime[:, :])
            nc.vector.tensor_tensor(mp_lo[:, :], mprime[:, :], mp_hi[:, :], ALU.subtract)

            # W_fcT tiles [128_j, NT, 128_v] bf16 via PE transpose
            wfc_nat = wkpool.tile([128, H], f32, tag="wfcnat")  # [v_p, j]
            nc.sync.dma_start(wfc_nat[:, :], W_fc[:, :])
            wfcT = cpool.tile([128, NT, V], bf16, tag="wfcT")
            for jt in range(NT):
                tp = ps_xp.tile([128, 128], f32, tag="xpp")
                nc.tensor.transpose(
                    tp[:, :], wfc_nat[:, jt * 128 : (jt + 1) * 128], ident_f32[:, :]
                )
                nc.vector.tensor_copy(wfcT[:, jt, :], tp[:, :])

            # b_fc broadcast [128_tok, V]
            bfc_row = wkpool.tile([1, V], f32, tag="bfcrow")
            nc.sync.dma_start(bfc_row[:, :], b_fc[:, :])
            bfc_bc = cpool.tile([128, V], f32, tag="bfcbc")
            nc.gpsimd.partition_broadcast(bfc_bc[:, :], bfc_row[:, :])

            # iota column [128, 1] for onehot compares
            iota_col = cpool.tile([128, 1], f32, tag="iota")
            nc.gpsimd.iota(iota_col[:, :], pattern=[[0, 1]], channel_multiplier=1,
                           allow_small_or_imprecise_dtypes=True)

            # hs ring buffers: [128, FC, NT, 2, B]; plane g=0 is h_hi,
            # g=1 is h_lo.  Slice [:, slot, it, :, :] is the contiguous
            # 32-col [h_hi|h_lo] moving operand for i-tile `it`.
            n_hs = 3
            hsbufs = [
                hspool.tile([128, FC, NT, 2, B], bf16, tag=f"hs{k}", name=f"hs{k}")
                for k in range(n_hs)
            ]
            # initial state (read as "step -1" = slot FC-1 of hsbufs[2])
            nc.vector.memset(
                hsbufs[n_hs - 1][:, FC - 1, :, :, :].rearrange("p a b c -> p (a b c)"),
                0.0,
            )

            # xp chunk double buffers [128, CH, 2, NT*B] bf16:
            # (step-in-chunk, {hi,lo}, jt*B+b).  Slice [:, si, :, :] is the
            # 128-col moving operand of the identity inject matmul.
            xp_bufs = [
                xppool.tile([128, CH, 2, NT * B], bf16, tag=f"xp{par}",
                            name=f"xp{par}")
                for par in range(2)
            ]

            onehot_cur = [None]

            def xp_prep(c):
                """Build onehot for chunk c (off the PE chain)."""
                xrow = wkpool.tile([1, CH * B], f32, tag="xrow")
                nc.sync.dma_start(
                    xrow[:, :],
                    x_tb.rearrange("(a t) b -> a (t b)", t=CH)[c : c + 1, :],
                )
                xbc = wkpool.tile([128, CH * B], f32, tag="xbc")
                nc.gpsimd.partition_broadcast(xbc[:, :], xrow[:, :])
                onehot = wkpool.tile([128, CH * B], bf16, tag="onehot")
                nc.vector.tensor_scalar(
                    onehot[:, :], xbc[:, :], iota_col[:, :], None, ALU.is_equal
                )
                onehot_cur[0] = onehot

            xp_psum_pend = {}

            def xp_mm(c, jt):
                """xp matmul for j-tile jt of chunk c: one psum group with
                both mp_hi and mp_lo product is NOT possible (different
                stationary) -> two matmuls accumulating into one psum."""
                onehot = onehot_cur[0]
                ps = ps_xp.tile([128, CH * B], f32, tag="xpp")
                nc.tensor.matmul(
                    ps[:, :], mp_hi[:, jt * 128 : (jt + 1) * 128], onehot[:, :],
                    start=True, stop=False, skip_group_check=True,
                )
                xp_psum_pend[jt] = ps
                return ps

            def xp_mm2(c, jt):
                ps = xp_psum_pend[jt]
                onehot = onehot_cur[0]
                nc.tensor.matmul(
                    ps[:, :], mp_lo[:, jt * 128 : (jt + 1) * 128], onehot[:, :],
                    start=False, stop=True, skip_group_check=True,
                )

            def xp_scatter(c, jt, piece):
                """Split psum for j-tile jt into bf16 hi/lo planes of the
                xp buffer (8 steps per piece)."""
                par = c % 2
                ps = xp_psum_pend[jt]
                q = CH // 4
                sl = slice(piece * q, (piece + 1) * q)
                dst = xp_bufs[par]
                dh = dst[:, sl, 0, jt * B : (jt + 1) * B]
                dl = dst[:, sl, 1, jt * B : (jt + 1) * B]
                sps = ps.rearrange("p (s b) -> p s b", s=CH)[:, sl, :]
                nc.vector.tensor_copy(dh, sps)
                nc.vector.tensor_tensor(dl, sps, dh, ALU.subtract)

            fc_state = {}

            def fc_part(hsbuf, s0, phase):
                """logits for FC steps starting at s0 from hsbuf; phases
                -1/0 repack (hi+lo add) + first 2 matmuls, 1 finishes."""
                hs_src = hsbuf
                if phase == -1:
                    fcbuf = wkpool.tile([128, NT * 128], bf16, tag="fcbuf")
                    fc_state[("buf", s0)] = fcbuf
                    nc.vector.tensor_tensor(
                        fcbuf.rearrange("p (g b s) -> p s g b", g=NT, b=B)[:, :, 0:2, :],
                        hs_src[:, :, 0:2, 0, :],
                        hs_src[:, :, 0:2, 1, :],
                        ALU.add,
                    )
                    return
                if phase == 0:
                    fcbuf = fc_state.pop(("buf", s0))
                    nc.vector.tensor_tensor(
                        fcbuf.rearrange("p (g b s) -> p s g b", g=NT, b=B)[:, :, 2:4, :],
                        hs_src[:, :, 2:4, 0, :],
                        hs_src[:, :, 2:4, 1, :],
                        ALU.add,
                    )
                    ps = ps_fc.tile([128, V], f32, tag="fcp")
                    fc_state[s0] = (fcbuf, ps)
                    for jt in (0, 1):
                        nc.tensor.matmul(
                            ps[:, :], fcbuf[:, jt * 128 : (jt + 1) * 128],
                            wfcT[:, jt, :],
                            start=(jt == 0), stop=False,
                            skip_group_check=(jt != 0),
                        )
                else:
                    fcbuf, ps = fc_state.pop(s0)
                    for jt in (2, 3):
                        nc.tensor.matmul(
                            ps[:, :], fcbuf[:, jt * 128 : (jt + 1) * 128],
                            wfcT[:, jt, :],
                            start=False, stop=(jt == 3),
                            skip_group_check=(jt != 3),
                        )
                    lg = wkpool.tile([128, V], f32, tag="logits")
                    nc.vector.tensor_tensor(lg[:, :], ps[:, :], bfc_bc[:, :], ALU.add)
                    # out[b, s0+s, v]: partitions p = b*FC + s
                    nc.sync.dma_start(out[:, s0 : s0 + FC, :], lg[:, :])

            def fc_phase(hsbuf, s0):
                fc_part(hsbuf, s0, -1)
                fc_part(hsbuf, s0, 0)
                fc_part(hsbuf, s0, 1)

            def prev_slice(s):
                if s == 0:
                    return hsbufs[n_hs - 1], FC - 1
                return hsbufs[((s - 1) // FC) % n_hs], (s - 1) % FC

            def rec_step_mms(c, s, psum, part):
                """Matmul pairs of step s.  part 0: ident + Wlo block;
                part 1: Whi block."""
                par = c % 2
                si = s - c * CH
                pb, pslot = prev_slice(s)
                if part == 0:
                    # identity inject of xp hi+lo: moving [128, (2,64)],
                    # stride-0 out over the hi/lo dim.
                    o = psum.rearrange("p (o c) -> p o c", o=1).broadcast_to(
                        [128, 2, NT * B])
                    nc.tensor.matmul(
                        o, ident_bf[:, :], xp_bufs[par][:, si, :, :],
                        start=True, stop=False, skip_group_check=True,
                    )
                    for it in range(NT):
                        mov = pb[:, pslot, it, 0, :]          # h_hi, 16 cols
                        for jt in range(NT):
                            nc.tensor.matmul(
                                psum[:, jt * B : (jt + 1) * B],
                                whh_lo[:, it, jt * 128 : (jt + 1) * 128],
                                mov,
                                start=False, stop=False, skip_group_check=True,
                            )
                else:
                    for it in range(NT):
                        mov = pb[:, pslot, it, :, :]          # [h_hi|h_lo], 32
                        for jt in range(NT):
                            last = (it == NT - 1) and (jt == NT - 1)
                            o = psum[:, jt * B : (jt + 1) * B].rearrange(
                                "p (o b) -> p o b", o=1).broadcast_to([128, 2, B])
                            nc.tensor.matmul(
                                o,
                                whh_hi[:, it, jt * 128 : (jt + 1) * 128],
                                mov,
                                start=False, stop=last,
                                skip_group_check=not last,
                            )

            def rec_step_tail(s, psum):
                """tanh chain: per-jt bf16 tanh (critical), then f32 tanh +
                lo subtract (off critical path)."""
                hb = hsbufs[(s // FC) % n_hs]
                slot = s % FC
                nc.scalar.activation(
                    hb[:, slot, :, 0, :],
                    psum.rearrange("p (jt b) -> p jt b", jt=NT),
                    AF.Tanh,
                )
                hT = spool.tile([128, NT * B], f32, tag="hT")
                nc.scalar.activation(hT[:, :], psum[:, :], AF.Tanh)
                nc.vector.tensor_tensor(
                    hb[:, slot, :, 1, :],
                    hT.rearrange("p (jt b) -> p jt b", jt=NT),
                    hb[:, slot, :, 0, :],
                    ALU.subtract,
                )

            # ---------------- main schedule ----------------
            def fillers(c, s):
                """xp chunk c+1 + fc matmuls, placed between the Wlo and
                Whi blocks of step s."""
                si = s - c * CH
                if c + 1 < n_chunks:
                    if si == 0:
                        xp_prep(c + 1)
                    elif si % 2 == 1 and si < 17:
                        k = si // 2  # 0..7
                        if k % 2 == 0:
                            xp_mm(c + 1, k // 2)
                        else:
                            xp_mm2(c + 1, k // 2)
                    if 4 <= si < 20:
                        jt_s, piece = (si - 4) // 4, (si - 4) % 4
                        if jt_s * 2 + 1 <= (si - 1) // 2:
                            xp_scatter(c + 1, jt_s, piece)
                if (s + 1) % FC == 5 and s + 1 >= FC * 2 - 3:
                    blk = (s + 8) // FC - 2
                    fc_part(hsbufs[blk % n_hs], blk * FC, -1)
                if (s + 1) % FC == 6 and s + 1 >= FC * 2 - 2:
                    blk = (s + 7) // FC - 2
                    fc_part(hsbufs[blk % n_hs], blk * FC, 0)
                if (s + 1) % FC == 0 and s + 1 >= FC * 2:
                    blk = (s + 1) // FC - 2
                    fc_part(hsbufs[blk % n_hs], blk * FC, 1)

            xp_prep(0)
            for jt in range(NT):
                xp_mm(0, jt)
                xp_mm2(0, jt)
                for piece in range(4):
                    xp_scatter(0, jt, piece)

            for c in range(n_chunks):
                for s in range(c * CH, (c + 1) * CH):
                    psum = ps_rec.tile([128, NT * B], f32, tag="rec")
                    fillers(c, s)
                    rec_step_mms(c, s, psum, 0)
                    rec_step_mms(c, s, psum, 1)
                    rec_step_tail(s, psum)
            # final two fc blocks
            for blk in (T // FC - 2, T // FC - 1):
                fc_phase(hsbufs[blk % n_hs], blk * FC)

    nc.finalize()
    return nc


_NC_CACHE = {}


def kernel(x, emb, W_ih, W_hh, b_ih, b_hh, W_fc, b_fc):
    from concourse.bass_utils import run_bass_kernel_spmd

    T_full = 1024
    x = np.asarray(x)
    emb = np.asarray(emb, dtype=np.float32)
    W_ih = np.asarray(W_ih, dtype=np.float32)
    W_hh = np.asarray(W_hh, dtype=np.float32)
    b_ih = np.asarray(b_ih, dtype=np.float32)
    b_hh = np.asarray(b_hh, dtype=np.float32)
    W_fc = np.asarray(W_fc, dtype=np.float32)
    b_fc = np.asarray(b_fc, dtype=np.float32)

    if "nc" not in _NC_CACHE:
        _NC_CACHE["nc"] = build(T_full)
    nc = _NC_CACHE["nc"]

    bias = (b_ih + b_hh).reshape(1, H).astype(np.float32)
    in_maps = []
    for core in range(8):
        xs = x[core * B : (core + 1) * B, :]          # [16, 1024]
        in_maps.append(dict(
            x_tb=np.ascontiguousarray(xs.T).astype(np.float32),
            emb=emb, W_ih=W_ih, W_hh=W_hh, bias=bias,
            W_fc=W_fc, b_fc=b_fc.reshape(1, V),
        ))
    res = run_bass_kernel_spmd(nc, in_maps, core_ids=list(range(8)))
    return np.concatenate([r["out"] for r in res.results], axis=0)
